# revision 12
# baseline (speedup 1.0000x reference)
"""MHA on 8 NeuronCores, v5: v2 phase discipline + surgical overlap fixes.

Core c owns token block c = (batch c//2, seq half c%2), 1024 tokens.
TRN2's activity governor duty-cycles the PE to 4/8 when the chip runs
too dense (v3/v4 fused schedules tripped it for 160-340us), so v5 keeps
v2's phase shape — attention is ACT(exp)-bound at ~95% with the PE near
70% — and fixes the overlap losses instead:

  - Projection order Q -> V -> K with staged SBUF reuse, so the K
    AllGathers (per d-chunk, staged inline) land exactly when attention
    needs them: first exp at ~90us instead of ~155us.
  - V staged via v_g tiles (2KB DMA lines) + DVE scatter into per-head
    v_t tiles, replacing 32K x 128B DMA descriptors that tripped the
    governor and starved the queues.
  - Attention qb-major (2 query halves x 16 heads, 8 groups of 2 key
    chunks), software-pipelined one scores-group ahead across unit
    boundaries so ACT never waits; PV's ones-column gives the softmax
    denominator; per-unit normalization via DRAM-broadcast reciprocal.
  - wo for the first token half runs inside the attention window (after
    the qb0 sweep); only the second half is tail work.

Communication: 8 pairwise K AllGathers (256KB in) + 4 pairwise V
AllGathers (512KB in), all overlapped.  bf16 matmuls, fp32 PSUM.
"""
import numpy as np
import ml_dtypes

import concourse.bass as bass
import concourse.bacc as bacc
import concourse.tile as tile
import concourse.mybir as mybir

N_CORES = 8
P = 128
B, S, D = 4, 2048, 1024
TOK = 1024  # my tokens
CD = D // P  # 8 chunks
QB = 512
NKC = S // P  # 16 key chunks
NU = 32  # units = 2 qb x 16 heads
F32 = mybir.dt.float32
BF16 = mybir.dt.bfloat16
EXP = mybir.ActivationFunctionType.Exp
PAIR_GROUPS = [[2 * i, 2 * i + 1] for i in range(4)]
# v_t position -> key chunk (vag quarters interleave the two cores)
KCS = [0, 1, 8, 9, 2, 3, 10, 11, 4, 5, 12, 13, 6, 7, 14, 15]

_CACHE = {}


def _n_excess_waits(nc):
    import json

    m = json.loads(nc.to_json_bytes())
    insts = [i for f in m["functions"] for b in f["blocks"] for i in b["instructions"]]
    return sum(
        1
        for i in insts
        if len((i.get("sync_info") or {}).get("on_wait", [])) >= 2
        and i.get("opcode") != "EventSemaphore"
    )


def _finish(nc):
    nc.compile()
    import bass_rust

    for _ in range(6):
        if _n_excess_waits(nc) == 0:
            break
        bass_rust.generate_event_semaphores(nc)
    assert _n_excess_waits(nc) == 0, "excess sync waits remain"
    nc.codegen_inst_isa_subclasses()
    return nc


def build_nc(scopes=False):
    nc = bacc.Bacc("TRN2", target_bir_lowering=False, debug=False, num_devices=N_CORES)

    xqT_d = nc.dram_tensor("xqT", [D, TOK], BF16, kind="ExternalInput").ap()
    xkT_d = nc.dram_tensor("xkT", [D, TOK], BF16, kind="ExternalInput").ap()
    xvT_d = nc.dram_tensor("xvT", [D, TOK], BF16, kind="ExternalInput").ap()
    wqkvT = nc.dram_tensor("wqkvT", [D, 3 * D], BF16, kind="ExternalInput").ap()
    woT = nc.dram_tensor("woT", [D, D], BF16, kind="ExternalInput").ap()
    out = nc.dram_tensor("out", [TOK, D], F32, kind="ExternalOutput").ap()

    kag_i = nc.dram_tensor("kag_i", [D, TOK], BF16).ap()
    kag_os = [nc.dram_tensor(f"kag_o{j}", [2, P, TOK], BF16).ap() for j in range(CD)]
    vag_i = nc.dram_tensor("vag_i", [TOK, D], BF16).ap()
    vag_os = [
        nc.dram_tensor(f"vag_o{q}", [2, TOK // 4, D], BF16).ap() for q in range(4)
    ]
    den_d = nc.dram_tensor("den_d", [NU, QB], F32).ap()
    recip_d = nc.dram_tensor("recip_d", [NU, QB], F32).ap()

    from contextlib import nullcontext

    def scope(name):
        return nc.named_scope(name) if scopes else nullcontext()

    AG_KW = dict(
        kind="AllGather", op=mybir.AluOpType.bypass, replica_groups=PAIR_GROUPS
    )

    with tile.TileContext(nc) as tc:
        # ---- persistent pools (bottom of the SBUF stack) ----------------
        qp = tc.alloc_tile_pool(name="qp", bufs=1)
        kst = tc.alloc_tile_pool(name="kst", bufs=1)
        vtp = tc.alloc_tile_pool(name="vtp", bufs=1)
        ltp = tc.alloc_tile_pool(name="ltp", bufs=1)
        vgp = tc.alloc_tile_pool(name="vgp", bufs=2)
        pgp = tc.alloc_tile_pool(name="pgp", bufs=6)
        arp = tc.alloc_tile_pool(name="arp", bufs=3)
        bcp = tc.alloc_tile_pool(name="bcp", bufs=2)
        smp = tc.alloc_tile_pool(name="smp", bufs=2)
        evp = tc.alloc_tile_pool(name="evp", bufs=4)
        obp = tc.alloc_tile_pool(name="obp", bufs=2)
        proj_ps = tc.alloc_tile_pool(name="proj_ps", bufs=2, space="PSUM")
        s_ps = tc.alloc_tile_pool(name="s_ps", bufs=2, space="PSUM")
        pv_ps = tc.alloc_tile_pool(name="pv_ps", bufs=2, space="PSUM")
        # prefetch slot for V inputs (released after proj_v)
        wvp = tc.alloc_tile_pool(name="wvp", bufs=1)
        xvp = tc.alloc_tile_pool(name="xvp", bufs=1)
        # Q-input slot, later reused for K inputs
        wqp = tc.alloc_tile_pool(name="wqp", bufs=1)
        xqp = tc.alloc_tile_pool(name="xqp", bufs=1)

        kT_s = [kst.tile([P, S], BF16, name=f"kTs_{j}") for j in range(CD)]
        qT_t = [qp.tile([P, TOK], BF16, name=f"qT_{i}") for i in range(CD)]
        v_t = [vtp.tile([P, NKC, 65], BF16, name=f"v_{h}") for h in range(16)]
        lts = [ltp.tile([P, TOK], BF16, name=f"lt_{i}") for i in range(CD)]

        # ---- loads: Q first, V prefetch alongside -----------------------
        with scope("load_qv"):
            wq_t, xq_t, wv_t, xv_t = [], [], [], []
            for j in range(CD):
                wq = wqp.tile([P, D], BF16, name=f"wq_{j}")
                nc.sync.dma_start(out=wq, in_=wqkvT[j * P : (j + 1) * P, 0:D])
                wq_t.append(wq)
                t = xqp.tile([P, TOK], BF16, name=f"xq_{j}")
                nc.sync.dma_start(out=t, in_=xqT_d[j * P : (j + 1) * P, :])
                xq_t.append(t)
            for j in range(CD):
                wv = wvp.tile([P, D], BF16, name=f"wv_{j}")
                nc.sync.dma_start(
                    out=wv, in_=wqkvT[j * P : (j + 1) * P, 2 * D : 3 * D]
                )
                wv_t.append(wv)
                t = xvp.tile([P, TOK], BF16, name=f"xv_{j}")
                nc.sync.dma_start(out=t, in_=xvT_d[j * P : (j + 1) * P, :])
                xv_t.append(t)

        # ---- proj Q -----------------------------------------------------
        with scope("proj_q"):
            for i in range(CD):
                for hf in range(2):
                    ps = proj_ps.tile([P, QB], F32, name="ps_p", tag="pp")
                    for j in range(CD):
                        nc.tensor.matmul(
                            ps,
                            wq_t[j][:, i * P : (i + 1) * P],
                            xq_t[j][:, hf * QB : (hf + 1) * QB],
                            start=(j == 0),
                            stop=(j == CD - 1),
                        )
                    nc.vector.tensor_copy(qT_t[i][:, hf * QB : (hf + 1) * QB], ps)
        xqp.release()
        wqp.release()

        # ---- K-input slot reuses the Q-input space ----------------------
        wkp = tc.alloc_tile_pool(name="wkp", bufs=1)
        xkp = tc.alloc_tile_pool(name="xkp", bufs=1)
        with scope("load_k"):
            wk_t, xk_t = [], []
            for j in range(CD):
                wk = wkp.tile([P, D], BF16, name=f"wk_{j}")
                nc.sync.dma_start(out=wk, in_=wqkvT[j * P : (j + 1) * P, D : 2 * D])
                wk_t.append(wk)
                t = xkp.tile([P, TOK], BF16, name=f"xk_{j}")
                nc.sync.dma_start(out=t, in_=xkT_d[j * P : (j + 1) * P, :])
                xk_t.append(t)

        # ---- proj V (AllGather per token quarter) -----------------------
        with scope("proj_v"):
            for c in range(CD):
                for hf in range(2):
                    ps = proj_ps.tile([P, QB], F32, name="ps_p", tag="pp")
                    for j in range(CD):
                        nc.tensor.matmul(
                            ps,
                            xv_t[j][:, c * P : (c + 1) * P],
                            wv_t[j][:, hf * QB : (hf + 1) * QB],
                            start=(j == 0),
                            stop=(j == CD - 1),
                        )
                    sb = evp.tile([P, QB], BF16, name="sb_e", tag="ev")
                    nc.vector.tensor_copy(sb, ps)
                    nc.sync.dma_start(
                        out=vag_i[c * P : (c + 1) * P, hf * QB : (hf + 1) * QB],
                        in_=sb,
                    )
                if c % 2 == 1:
                    q = c // 2
                    nc.gpsimd.collective_compute(
                        ins=[vag_i[q * (TOK // 4) : (q + 1) * (TOK // 4), :]],
                        outs=[vag_os[q][:]],
                        **AG_KW,
                    )

        def k_chunk(i):
            with scope("proj_k"):
                for hf in range(2):
                    ps = proj_ps.tile([P, QB], F32, name="ps_p", tag="pp")
                    for j in range(CD):
                        nc.tensor.matmul(
                            ps,
                            wk_t[j][:, i * P : (i + 1) * P],
                            xk_t[j][:, hf * QB : (hf + 1) * QB],
                            start=(j == 0),
                            stop=(j == CD - 1),
                        )
                    sb = evp.tile([P, QB], BF16, name="sb_e", tag="ev")
                    nc.vector.tensor_copy(sb, ps)
                    nc.sync.dma_start(
                        out=kag_i[i * P : (i + 1) * P, hf * QB : (hf + 1) * QB],
                        in_=sb,
                    )
                nc.gpsimd.collective_compute(
                    ins=[kag_i[i * P : (i + 1) * P, :]], outs=[kag_os[i][:]], **AG_KW
                )
                nc.sync.dma_start(out=kT_s[i][:, 0:TOK], in_=kag_os[i][0])
                nc.sync.dma_start(out=kT_s[i][:, TOK:S], in_=kag_os[i][1])

        # K chunks 0-1 first so their evictions/AGs aren't queued behind
        # the v_g staging; then v_g (gpsimd scatter), then K 2-7.
        k_chunk(0)
        k_chunk(1)

        # ---- v_g staging: 2KB DMA lines + gpsimd scatter into v_t -------
        with scope("vt_stage"):
            for h in range(16):
                nc.vector.memset(v_t[h][:, :, 64:65], 1.0)
            for q in range(4):
                for half in range(2):
                    vg = vgp.tile([P, 2, D], BF16, name="vg", tag="vg")
                    nc.sync.dma_start(
                        out=vg,
                        in_=vag_os[q][half].rearrange("(kc p) d -> p kc d", p=P),
                    )
                    pos = 4 * q + 2 * half
                    for h in range(16):
                        nc.gpsimd.tensor_copy(
                            v_t[h][:, pos : pos + 2, 0:64],
                            vg[:, :, 64 * h : 64 * h + 64],
                        )

        for i in range(2, CD):
            k_chunk(i)
        xkp.release()
        wkp.release()
        xvp.release()
        wvp.release()

        # ---- wo tiles (reuse released input space) ----------------------
        wop = tc.alloc_tile_pool(name="wop", bufs=1)
        wo_t = []
        for sc in range(CD):
            wt = wop.tile([P, D], BF16, name=f"wo_{sc}")
            nc.sync.dma_start(out=wt, in_=woT[sc * P : (sc + 1) * P, :])
            wo_t.append(wt)

        def emit_wo(t_i, hf):
            with scope("wo"):
                ps = proj_ps.tile([P, QB], F32, name="ps_p", tag="pp")
                for sc in range(CD):
                    nc.tensor.matmul(
                        ps,
                        lts[sc][:, t_i * P : (t_i + 1) * P],
                        wo_t[sc][:, hf * QB : (hf + 1) * QB],
                        start=(sc == 0),
                        stop=(sc == CD - 1),
                    )
                ob = obp.tile([P, QB], F32, name="ob", tag="ob")
                nc.vector.tensor_copy(ob, ps)
                nc.sync.dma_start(
                    out=out[t_i * P : (t_i + 1) * P, hf * QB : (hf + 1) * QB],
                    in_=ob,
                )

        # ---- attention: qb-major, one scores-group lookahead ------------
        units = [(qb, h) for qb in range(2) for h in range(16)]
        steps = [(ui, g) for ui in range(NU) for g in range(8)]
        pvs = {}

        def emit_scores(step):
            ui, g = step
            qb, h = units[ui]
            r = slice(64 * (h % 2), 64 * (h % 2) + 64)
            qs = slice(qb * QB, (qb + 1) * QB)
            sg = s_ps.tile([P, 2, QB], F32, name="sg", tag="sg")
            for jj in range(2):
                kc = KCS[2 * g + jj]
                nc.tensor.matmul(
                    sg[:, jj, :],
                    kT_s[h // 2][r, kc * P : (kc + 1) * P],
                    qT_t[h // 2][r, qs],
                    start=True,
                    stop=True,
                )
            return sg

        def emit_exp(step, sg):
            pg = pgp.tile([P, 2, QB], BF16, name="pg", tag="pg")
            nc.scalar.activation(pg, sg, EXP, scale=0.125)
            return pg

        def emit_pv(step, pg):
            ui, g = step
            qb, h = units[ui]
            if g == 0:
                pvs[ui] = pv_ps.tile([65, QB], F32, name="pv", tag="pv")
            for jj in range(2):
                pos = 2 * g + jj
                nc.tensor.matmul(
                    pvs[ui],
                    v_t[h][:, pos, :],
                    pg[:, jj, :],
                    start=(pos == 0),
                    stop=(pos == NKC - 1),
                )
            if g == 7:
                finish_unit(ui)

        def finish_unit(ui):
            qb, h = units[ui]
            pv = pvs.pop(ui)
            with scope("norm"):
                r = slice(64 * (h % 2), 64 * (h % 2) + 64)
                qs = slice(qb * QB, (qb + 1) * QB)
                araw = arp.tile([65, QB], F32, name="araw", tag="ar")
                nc.vector.tensor_copy(araw, pv)
                nc.sync.dma_start(out=den_d[ui : ui + 1, :], in_=araw[64:65, :])
                dsq = smp.tile([64, 8], F32, name="dsq", tag="d")
                nc.sync.dma_start(
                    out=dsq,
                    in_=bass.AP(
                        tensor=den_d.tensor, offset=ui * QB, ap=[[8, 64], [1, 8]]
                    ),
                )
                rsq = smp.tile([64, 8], F32, name="rsq", tag="r")
                nc.vector.reciprocal(rsq, dsq)
                nc.sync.dma_start(
                    out=bass.AP(
                        tensor=recip_d.tensor, offset=ui * QB, ap=[[8, 64], [1, 8]]
                    ),
                    in_=rsq,
                )
                bc = bcp.tile([64, QB], F32, name="bc", tag="bc")
                nc.sync.dma_start(
                    out=bc,
                    in_=bass.AP(
                        tensor=recip_d.tensor, offset=ui * QB, ap=[[0, 64], [1, QB]]
                    ),
                )
                nc.vector.tensor_mul(lts[h // 2][r, qs], araw[0:64, :], bc)
            # wo for the first token half, paced across the qb1 sweep
            if 16 <= ui < 24:
                emit_wo((ui - 16) // 2, (ui - 16) % 2)

        with scope("attn"):
            sg_cur = emit_scores(steps[0])
            for i, step in enumerate(steps):
                pg = emit_exp(step, sg_cur)
                if i + 1 < len(steps):
                    sg_cur = emit_scores(steps[i + 1])
                emit_pv(step, pg)

        # ---- tail: wo for the second token half -------------------------
        with scope("wo_tail"):
            for t_i in range(4, CD):
                for hf in range(2):
                    emit_wo(t_i, hf)

        for p in (
            wop, obp, evp, smp, bcp, arp, pgp, vgp, ltp, vtp, kst, qp,
        ):
            p.release()
        for p in (pv_ps, s_ps, proj_ps):
            p.release()

    return _finish(nc)


def _get_nc(scopes=False):
    key = ("nc", scopes)
    if key not in _CACHE:
        _CACHE[key] = build_nc(scopes)
    return _CACHE[key]


def make_in_maps(query, key, value, wq, wk, wv, wo):
    qf = np.asarray(query, np.float32).reshape(B * S, D)
    kf = np.asarray(key, np.float32).reshape(B * S, D)
    vf = np.asarray(value, np.float32).reshape(B * S, D)
    wqkvT = np.ascontiguousarray(
        np.concatenate([np.asarray(wq), np.asarray(wk), np.asarray(wv)], 0).T
    ).astype(ml_dtypes.bfloat16)
    woT_h = np.ascontiguousarray(np.asarray(wo).T).astype(ml_dtypes.bfloat16)
    in_maps = []
    for c in range(N_CORES):
        sl = slice(c * TOK, (c + 1) * TOK)
        in_maps.append(
            {
                "xqT": np.ascontiguousarray(qf[sl].T).astype(ml_dtypes.bfloat16),
                "xkT": np.ascontiguousarray(kf[sl].T).astype(ml_dtypes.bfloat16),
                "xvT": np.ascontiguousarray(vf[sl].T).astype(ml_dtypes.bfloat16),
                "wqkvT": wqkvT,
                "woT": woT_h,
            }
        )
    return in_maps


def assemble(results):
    blocks = [results[c]["out"] for c in range(N_CORES)]
    return np.concatenate(blocks, 0).reshape(B, S, D).astype(np.float32)


def kernel(query, key, value, mask, wq, wk, wv, wo):
    # mask is all-False in this problem: softmax without masking.
    nc = _get_nc()
    in_maps = make_in_maps(query, key, value, wq, wk, wv, wo)
    from concourse.bass_utils import run_bass_kernel_spmd

    res = run_bass_kernel_spmd(nc, in_maps, list(range(N_CORES)))
    return assemble(res.results)


# revision 14
# speedup vs baseline: 1.0966x; 1.0966x over previous
"""MHA on 8 NeuronCores, v7: v2's attention + earlier start.

Core c owns token block c = (batch c//2, seq half c%2), 1024 tokens.
TRN2's activity governor duty-cycles the PE to 4/8 under dense mixed
activity; v2's attention profile (PE ~88%, ACT ~84%, low DMA, gpsimd
idle) is proven to run unclamped, so v7 keeps that attention shape
exactly and only moves its start earlier:

  - K+V inputs prefetch at t=0; proj K first with per-d-chunk pairwise
    AllGather + kT staging inline (landed long before attention);
    proj V next (AllGather per token quarter); Q inputs reuse K's SBUF
    slot and project last.  First exp at ~120us instead of ~155us.
  - Attention (16 heads x 2 query halves, key-chunk groups of
    3,3,3,3,3,1): scores (PE) -> exp (ACT) -> PV (PE, ones-column in V
    gives the softmax denominator), software-pipelined at depth 2
    (exp(i); scores(i+2); PV(i)) so ACT never waits at boundaries.
    v_t loads stay v2-style per-head rearrange DMAs, one head ahead.
  - Per-head normalization via DRAM-broadcast reciprocal into wo-ready
    lts tiles; wo projection in the tail.
  - PSUM pools are phase-sequential: proj (2 banks) -> attention
    (6 scores + 2 PV) -> wo (2).

Communication: 8 pairwise K AllGathers (256KB in) + 4 pairwise V
AllGathers (512KB in), all overlapped.  bf16 matmuls, fp32 PSUM.
"""
import numpy as np
import ml_dtypes

import concourse.bass as bass
import concourse.bacc as bacc
import concourse.tile as tile
import concourse.mybir as mybir

N_CORES = 8
P = 128
B, S, D = 4, 2048, 1024
TOK = 1024  # my tokens
CD = D // P  # 8 chunks
QB = 512
NKC = S // P  # 16 key chunks
F32 = mybir.dt.float32
BF16 = mybir.dt.bfloat16
EXP = mybir.ActivationFunctionType.Exp
PAIR_GROUPS = [[2 * i, 2 * i + 1] for i in range(4)]
# v_t position -> key chunk (vag quarters interleave the two cores)
KCS = [0, 1, 8, 9, 2, 3, 10, 11, 4, 5, 12, 13, 6, 7, 14, 15]
GROUPS = [(0, 3), (3, 6), (6, 9), (9, 12), (12, 15), (15, 16)]

_CACHE = {}


def _n_excess_waits(nc):
    import json

    m = json.loads(nc.to_json_bytes())
    insts = [i for f in m["functions"] for b in f["blocks"] for i in b["instructions"]]
    return sum(
        1
        for i in insts
        if len((i.get("sync_info") or {}).get("on_wait", [])) >= 2
        and i.get("opcode") != "EventSemaphore"
    )


def _finish(nc):
    nc.compile()
    import bass_rust

    for _ in range(6):
        if _n_excess_waits(nc) == 0:
            break
        bass_rust.generate_event_semaphores(nc)
    assert _n_excess_waits(nc) == 0, "excess sync waits remain"
    nc.codegen_inst_isa_subclasses()
    return nc


def build_nc(scopes=False):
    nc = bacc.Bacc("TRN2", target_bir_lowering=False, debug=False, num_devices=N_CORES)

    xqT_d = nc.dram_tensor("xqT", [D, TOK], BF16, kind="ExternalInput").ap()
    xkT_d = nc.dram_tensor("xkT", [D, TOK], BF16, kind="ExternalInput").ap()
    xvT_d = nc.dram_tensor("xvT", [D, TOK], BF16, kind="ExternalInput").ap()
    wqkvT = nc.dram_tensor("wqkvT", [D, 3 * D], BF16, kind="ExternalInput").ap()
    woT = nc.dram_tensor("woT", [D, D], BF16, kind="ExternalInput").ap()
    out = nc.dram_tensor("out", [TOK, D], F32, kind="ExternalOutput").ap()

    kag_i = nc.dram_tensor("kag_i", [D, TOK], BF16).ap()
    kag_os = [nc.dram_tensor(f"kag_o{j}", [2, P, TOK], BF16).ap() for j in range(CD)]
    vag_i = nc.dram_tensor("vag_i", [TOK, D], BF16).ap()
    vag_os = [
        nc.dram_tensor(f"vag_o{q}", [2, TOK // 4, D], BF16).ap() for q in range(4)
    ]
    den_d = nc.dram_tensor("den_d", [16, TOK], F32).ap()
    recip_d = nc.dram_tensor("recip_d", [16, TOK], F32).ap()

    from contextlib import nullcontext

    def scope(name):
        return nc.named_scope(name) if scopes else nullcontext()

    AG_KW = dict(
        kind="AllGather", op=mybir.AluOpType.bypass, replica_groups=PAIR_GROUPS
    )

    with tile.TileContext(nc) as tc:
        # ---- persistent pools -------------------------------------------
        qp = tc.alloc_tile_pool(name="qp", bufs=1)
        kst = tc.alloc_tile_pool(name="kst", bufs=1)
        ltp = tc.alloc_tile_pool(name="ltp", bufs=1)
        vp = tc.alloc_tile_pool(name="vp", bufs=3)
        pt = tc.alloc_tile_pool(name="pt", bufs=4)
        at = tc.alloc_tile_pool(name="at", bufs=3)
        sm = tc.alloc_tile_pool(name="sm", bufs=2)
        evp = tc.alloc_tile_pool(name="evp", bufs=4)
        obp = tc.alloc_tile_pool(name="obp", bufs=3)
        # PSUM is phase-sequential: proj -> (scores+pv) -> wo
        proj_ps = tc.alloc_tile_pool(name="proj_ps", bufs=2, space="PSUM")
        # input slots: V prefetch + K (reused for Q later)
        wvp = tc.alloc_tile_pool(name="wvp", bufs=1)
        xvp = tc.alloc_tile_pool(name="xvp", bufs=1)
        wkp = tc.alloc_tile_pool(name="wkp", bufs=1)
        xkp = tc.alloc_tile_pool(name="xkp", bufs=1)

        kT_s = [kst.tile([P, S], BF16, name=f"kTs_{j}") for j in range(CD)]
        qT_t = [qp.tile([P, TOK], BF16, name=f"qT_{i}") for i in range(CD)]
        lts = [ltp.tile([P, TOK], BF16, name=f"lt_{i}") for i in range(CD)]

        # ---- loads: K first, V prefetch alongside -----------------------
        with scope("load_kv"):
            wk_t, xk_t, wv_t, xv_t = [], [], [], []
            for j in range(CD):
                wk = wkp.tile([P, D], BF16, name=f"wk_{j}")
                nc.sync.dma_start(out=wk, in_=wqkvT[j * P : (j + 1) * P, D : 2 * D])
                wk_t.append(wk)
                t = xkp.tile([P, TOK], BF16, name=f"xk_{j}")
                nc.sync.dma_start(out=t, in_=xkT_d[j * P : (j + 1) * P, :])
                xk_t.append(t)
            for j in range(CD):
                wv = wvp.tile([P, D], BF16, name=f"wv_{j}")
                nc.sync.dma_start(
                    out=wv, in_=wqkvT[j * P : (j + 1) * P, 2 * D : 3 * D]
                )
                wv_t.append(wv)
                t = xvp.tile([P, TOK], BF16, name=f"xv_{j}")
                nc.sync.dma_start(out=t, in_=xvT_d[j * P : (j + 1) * P, :])
                xv_t.append(t)

        # ---- proj K (AllGather + kT staging per d-chunk) ----------------
        with scope("proj_k"):
            for i in range(CD):
                for hf in range(2):
                    ps = proj_ps.tile([P, QB], F32, name="ps_p", tag="pp")
                    for j in range(CD):
                        nc.tensor.matmul(
                            ps,
                            wk_t[j][:, i * P : (i + 1) * P],
                            xk_t[j][:, hf * QB : (hf + 1) * QB],
                            start=(j == 0),
                            stop=(j == CD - 1),
                        )
                    sb = evp.tile([P, QB], BF16, name="sb_e", tag="ev")
                    nc.vector.tensor_copy(sb, ps)
                    nc.sync.dma_start(
                        out=kag_i[i * P : (i + 1) * P, hf * QB : (hf + 1) * QB],
                        in_=sb,
                    )
                nc.gpsimd.collective_compute(
                    ins=[kag_i[i * P : (i + 1) * P, :]], outs=[kag_os[i][:]], **AG_KW
                )
                nc.sync.dma_start(out=kT_s[i][:, 0:TOK], in_=kag_os[i][0])
                nc.sync.dma_start(out=kT_s[i][:, TOK:S], in_=kag_os[i][1])
        xkp.release()
        wkp.release()

        # ---- Q-input slot reuses the K-input space ----------------------
        wqp = tc.alloc_tile_pool(name="wqp", bufs=1)
        xqp = tc.alloc_tile_pool(name="xqp", bufs=1)
        with scope("load_q"):
            wq_t, xq_t = [], []
            for j in range(CD):
                wq = wqp.tile([P, D], BF16, name=f"wq_{j}")
                nc.sync.dma_start(out=wq, in_=wqkvT[j * P : (j + 1) * P, 0:D])
                wq_t.append(wq)
                t = xqp.tile([P, TOK], BF16, name=f"xq_{j}")
                nc.sync.dma_start(out=t, in_=xqT_d[j * P : (j + 1) * P, :])
                xq_t.append(t)

        # ---- proj V (AllGather per token quarter) -----------------------
        with scope("proj_v"):
            for c in range(CD):
                for hf in range(2):
                    ps = proj_ps.tile([P, QB], F32, name="ps_p", tag="pp")
                    for j in range(CD):
                        nc.tensor.matmul(
                            ps,
                            xv_t[j][:, c * P : (c + 1) * P],
                            wv_t[j][:, hf * QB : (hf + 1) * QB],
                            start=(j == 0),
                            stop=(j == CD - 1),
                        )
                    sb = evp.tile([P, QB], BF16, name="sb_e", tag="ev")
                    nc.vector.tensor_copy(sb, ps)
                    nc.sync.dma_start(
                        out=vag_i[c * P : (c + 1) * P, hf * QB : (hf + 1) * QB],
                        in_=sb,
                    )
                if c % 2 == 1:
                    q = c // 2
                    nc.gpsimd.collective_compute(
                        ins=[vag_i[q * (TOK // 4) : (q + 1) * (TOK // 4), :]],
                        outs=[vag_os[q][:]],
                        **AG_KW,
                    )

        # ---- proj Q -----------------------------------------------------
        with scope("proj_q"):
            for i in range(CD):
                for hf in range(2):
                    ps = proj_ps.tile([P, QB], F32, name="ps_p", tag="pp")
                    for j in range(CD):
                        nc.tensor.matmul(
                            ps,
                            wq_t[j][:, i * P : (i + 1) * P],
                            xq_t[j][:, hf * QB : (hf + 1) * QB],
                            start=(j == 0),
                            stop=(j == CD - 1),
                        )
                    nc.vector.tensor_copy(qT_t[i][:, hf * QB : (hf + 1) * QB], ps)
        xqp.release()
        wqp.release()
        xvp.release()
        wvp.release()

        # ---- wo prefetch (reuses released input space) ------------------
        wop = tc.alloc_tile_pool(name="wop", bufs=1)
        wo_t = []
        for sc in range(CD):
            wt = wop.tile([P, D], BF16, name=f"wo_{sc}")
            nc.sync.dma_start(out=wt, in_=woT[sc * P : (sc + 1) * P, :])
            wo_t.append(wt)

        # ---- attention: v2 structure, depth-2 software pipeline ---------
        proj_ps.release()
        s_ps = tc.alloc_tile_pool(name="s_ps", bufs=2, space="PSUM")
        pv_ps = tc.alloc_tile_pool(name="pv_ps", bufs=2, space="PSUM")

        vts, araws, pvs = {}, {}, {}

        def load_head(h):
            if h in vts:
                return
            v_tile = vp.tile([P, NKC, 65], BF16, name="v_t", tag="vp")
            for q in range(4):
                for half in range(2):
                    vsrc = vag_os[q][half, :, 64 * h : 64 * h + 64]
                    nc.sync.dma_start(
                        out=v_tile[:, 4 * q + 2 * half : 4 * q + 2 * half + 2, 0:64],
                        in_=vsrc.rearrange("(kc p) d -> p kc d", p=P),
                    )
            nc.vector.memset(v_tile[:, :, 64:65], 1.0)
            vts[h] = v_tile

        steps = [
            (h, qb, gi)
            for h in range(16)
            for qb in range(TOK // QB)
            for gi in range(len(GROUPS))
        ]

        def emit_scores(step):
            h, qb, gi = step
            if qb == 0 and gi == 0:
                load_head(h)
                if h + 1 < 16:
                    load_head(h + 1)
                araws[h] = at.tile([65, TOK], F32, name="a_raw", tag="at")
            g0, g1 = GROUPS[gi]
            if gi == 0:
                pvs[(h, qb)] = pv_ps.tile([65, QB], F32, name="pv", tag="pv_ps")
            r = slice(64 * (h % 2), 64 * (h % 2) + 64)
            qs = slice(qb * QB, (qb + 1) * QB)
            sg = s_ps.tile([P, 3, QB], F32, name="sg", tag="s_ps")
            for pos in range(g0, g1):
                kc = KCS[pos]
                nc.tensor.matmul(
                    sg[:, pos - g0, :],
                    kT_s[h // 2][r, kc * P : (kc + 1) * P],
                    qT_t[h // 2][r, qs],
                    start=True,
                    stop=True,
                )
            return sg

        def emit_exp(step, sg):
            h, qb, gi = step
            g0, g1 = GROUPS[gi]
            n = g1 - g0
            pg = pt.tile([P, 3, QB], BF16, name="pg", tag="pt")
            nc.scalar.activation(pg[:, 0:n, :], sg[:, 0:n, :], EXP, scale=0.125)
            return pg

        def emit_pv(step, pg):
            h, qb, gi = step
            g0, g1 = GROUPS[gi]
            for pos in range(g0, g1):
                nc.tensor.matmul(
                    pvs[(h, qb)],
                    vts[h][:, pos, :],
                    pg[:, pos - g0, :],
                    start=(pos == 0),
                    stop=(pos == NKC - 1),
                )
            if g1 == NKC:
                qs = slice(qb * QB, (qb + 1) * QB)
                nc.vector.tensor_copy(araws[h][:, qs], pvs[(h, qb)])
                if qb == TOK // QB - 1:
                    finish_head(h)

        def finish_head(h):
            with scope(f"norm"):
                a_raw = araws.pop(h)
                if h >= 2:
                    vts.pop(h - 2, None)
                nc.sync.dma_start(out=den_d[h : h + 1, :], in_=a_raw[64:65, :])
                dsq = sm.tile([64, 16], F32, name="dsq", tag="smd")
                nc.sync.dma_start(
                    out=dsq,
                    in_=bass.AP(
                        tensor=den_d.tensor, offset=h * TOK, ap=[[16, 64], [1, 16]]
                    ),
                )
                rsq = sm.tile([64, 16], F32, name="rsq", tag="smr")
                nc.vector.reciprocal(rsq, dsq)
                nc.sync.dma_start(
                    out=bass.AP(
                        tensor=recip_d.tensor, offset=h * TOK, ap=[[16, 64], [1, 16]]
                    ),
                    in_=rsq,
                )
                bc = at.tile([64, TOK], F32, name="bc", tag="at2")
                nc.sync.dma_start(
                    out=bc,
                    in_=bass.AP(
                        tensor=recip_d.tensor, offset=h * TOK, ap=[[0, 64], [1, TOK]]
                    ),
                )
                rr = slice(64 * (h % 2), 64 * (h % 2) + 64)
                nc.vector.tensor_mul(lts[h // 2][rr, :], a_raw[0:64, :], bc)

        with scope("attn"):
            sgs = [emit_scores(steps[0]), emit_scores(steps[1])]
            for i, step in enumerate(steps):
                pg = emit_exp(step, sgs[i % 2])
                if i + 2 < len(steps):
                    sgs[i % 2] = emit_scores(steps[i + 2])
                emit_pv(step, pg)

        # ---- wo tail ----------------------------------------------------
        pv_ps.release()
        s_ps.release()
        wo_ps = tc.alloc_tile_pool(name="wo_ps", bufs=3, space="PSUM")
        with scope("wo"):
            for t_i in range(CD):
                ps3 = wo_ps.tile([P, D], F32, name="ps3", tag="ps3")
                for sc in range(CD):
                    for hh in range(2):
                        nc.tensor.matmul(
                            ps3[:, hh * QB : (hh + 1) * QB],
                            lts[sc][:, t_i * P : (t_i + 1) * P],
                            wo_t[sc][:, hh * QB : (hh + 1) * QB],
                            start=(sc == 0),
                            stop=(sc == CD - 1),
                        )
                ob = obp.tile([P, D], F32, name="ob", tag="ob")
                nc.vector.tensor_copy(ob, ps3)
                nc.sync.dma_start(out=out[t_i * P : (t_i + 1) * P, :], in_=ob)

        for p in (wop, obp, evp, sm, at, pt, vp, ltp, kst, qp):
            p.release()
        wo_ps.release()

    return _finish(nc)


def _get_nc(scopes=False):
    key = ("nc", scopes)
    if key not in _CACHE:
        _CACHE[key] = build_nc(scopes)
    return _CACHE[key]


def make_in_maps(query, key, value, wq, wk, wv, wo):
    qf = np.asarray(query, np.float32).reshape(B * S, D)
    kf = np.asarray(key, np.float32).reshape(B * S, D)
    vf = np.asarray(value, np.float32).reshape(B * S, D)
    wqkvT = np.ascontiguousarray(
        np.concatenate([np.asarray(wq), np.asarray(wk), np.asarray(wv)], 0).T
    ).astype(ml_dtypes.bfloat16)
    woT_h = np.ascontiguousarray(np.asarray(wo).T).astype(ml_dtypes.bfloat16)
    in_maps = []
    for c in range(N_CORES):
        sl = slice(c * TOK, (c + 1) * TOK)
        in_maps.append(
            {
                "xqT": np.ascontiguousarray(qf[sl].T).astype(ml_dtypes.bfloat16),
                "xkT": np.ascontiguousarray(kf[sl].T).astype(ml_dtypes.bfloat16),
                "xvT": np.ascontiguousarray(vf[sl].T).astype(ml_dtypes.bfloat16),
                "wqkvT": wqkvT,
                "woT": woT_h,
            }
        )
    return in_maps


def assemble(results):
    blocks = [results[c]["out"] for c in range(N_CORES)]
    return np.concatenate(blocks, 0).reshape(B, S, D).astype(np.float32)


def kernel(query, key, value, mask, wq, wk, wv, wo):
    # mask is all-False in this problem: softmax without masking.
    nc = _get_nc()
    in_maps = make_in_maps(query, key, value, wq, wk, wv, wo)
    from concourse.bass_utils import run_bass_kernel_spmd

    res = run_bass_kernel_spmd(nc, in_maps, list(range(N_CORES)))
    return assemble(res.results)
